# revision 1
# baseline (speedup 1.0000x reference)
"""Trainium2 Bass kernel for nn_Cross_AgentAttention.

Data-parallel over batch B=8 across 8 NeuronCores; params replicated.

Per-core algorithm (feature-major (c, n) layout, exploiting that
q = guidmap @ q_w + q_b is rank-1):
  - v = x @ Wv via fp32r matmuls (TF32-grade, full PE rate)
  - agent->kv attention collapses to kw[h,i] = w_h . k_h[i], computed as
    x @ Mkw (folded on host); logits = scale*gbar_a*kw[h,i] + PB, where
    row-constant terms cancel in softmax.
  - query->agent attention collapses to a rank-1 logit map
    lq[i,(h,a)] = g_i * u[(h,a)] (+r) + ABt.
  - depthwise 3x3 conv = 9 shifted diagonal matmuls accumulated into the
    same PSUM tile as the attention-output matmul.
"""
import numpy as np
import ml_dtypes

import concourse.bass as bass
import concourse.bacc as bacc
import concourse.mybir as mybir
from concourse.tile import TileContext
from concourse.bass_utils import run_bass_kernel_spmd

F32 = mybir.dt.float32
F32R = mybir.dt.float32r
BF16 = mybir.dt.bfloat16
BF = ml_dtypes.bfloat16

DIM = 256
HEADS = 8
AGENT = 16
H = W = 64
B = 8
N = H * W                 # 4096
HD = DIM // HEADS         # 32
SCALE = HD ** -0.5
PS = 4
NT = 8                    # n-tiles of 512
NTW = N // NT             # 512
NCH = 32                  # n-chunks of 128
ROWS_PER_NT = NTW // W    # 8 image rows per n-tile

AL = mybir.AluOpType
AF = mybir.ActivationFunctionType


# ----------------------------------------------------------------------------
# host precompute
# ----------------------------------------------------------------------------

def _bilinear_matrix(n_in, n_out):
    U = np.zeros((n_out, n_in), dtype=np.float64)
    s = n_in / n_out
    for o in range(n_out):
        x = (o + 0.5) * s - 0.5
        x0 = int(np.floor(x))
        t = x - x0
        for i, wt in ((x0, 1.0 - t), (x0 + 1, t)):
            ic = min(max(i, 0), n_in - 1)
            U[o, ic] += wt
    return U.astype(np.float32)


def _host_precompute(kv_w, kv_b, q_w, q_b, proj_w, proj_b, dwc_w, dwc_b,
                     an_bias, na_bias, ah_bias, aw_bias, ha_bias, wa_bias):
    c = DIM
    w = q_w[0]
    beta = q_b
    U = _bilinear_matrix(PS, H)

    # logits-matmul constant operands: logits = LOG^T @ RHS with
    # LOG rows = [Eg-gbar(8, device) | EgC(8) | an_tbl(16) | ahT(64) | awT(64)]
    # RHS rows = [kw(8, device) | kbeta(8, device) | UU(16) | Yind(64) | Xind(64)]
    an_tbl = an_bias.reshape(HEADS * AGENT, PS * PS).T.astype(np.float32)      # (16, 128)
    UU = np.einsum("yr,xc->rcyx", U, U).reshape(PS * PS, N).astype(np.float32)  # (16, 4096)
    ahT = ah_bias[0][..., 0].reshape(HEADS * AGENT, H).T.astype(np.float32)    # (64, 128)
    awT = aw_bias[0][:, :, 0, :].reshape(HEADS * AGENT, W).T.astype(np.float32)
    Yind = np.kron(np.eye(H, dtype=np.float32), np.ones((1, W), np.float32))    # (64, 4096)
    Xind = np.concatenate([np.eye(W, dtype=np.float32)] * H, axis=1)            # (64, 4096)

    na_up = np.einsum("yr,harc,xc->hayx", U, na_bias.reshape(HEADS, AGENT, PS, PS), U)
    ab = na_up.reshape(HEADS, AGENT, N).transpose(0, 2, 1)
    ab = ab + (ha_bias[0] + wa_bias[0]).reshape(HEADS, N, AGENT)
    ABt = ab.transpose(1, 0, 2).reshape(N, HEADS * AGENT).astype(np.float32)

    wk = kv_w[:, :c]
    Mkw = np.stack([(wk[:, h*HD:(h+1)*HD] * w[None, h*HD:(h+1)*HD]).sum(1)
                    for h in range(HEADS)], axis=1)
    Mkb = np.stack([(wk[:, h*HD:(h+1)*HD] * beta[None, h*HD:(h+1)*HD]).sum(1)
                    for h in range(HEADS)], axis=1)
    MM = np.concatenate([Mkw, Mkb], axis=1).astype(np.float32)      # (256, 16)

    hw2 = np.array([(w[h*HD:(h+1)*HD]**2).sum() for h in range(HEADS)], np.float32)
    wb = np.array([(w[h*HD:(h+1)*HD]*beta[h*HD:(h+1)*HD]).sum() for h in range(HEADS)], np.float32)
    bb = np.array([(beta[h*HD:(h+1)*HD]**2).sum() for h in range(HEADS)], np.float32)
    # qrows: [s*hw2 | s*wb | s*bb] repeated per agent -> (1, 384)
    qrows = np.concatenate([np.repeat(SCALE * hw2, AGENT),
                            np.repeat(SCALE * wb, AGENT),
                            np.repeat(SCALE * bb, AGENT)])[None, :].astype(np.float32)

    Wv = kv_w[:, c:].astype(np.float32)                              # (256, 256)
    bv = kv_b[c:].astype(np.float32)

    headmask = np.zeros((HEADS * AGENT, c), np.float32)
    for h in range(HEADS):
        headmask[h*AGENT:(h+1)*AGENT, h*HD:(h+1)*HD] = 1.0

    # EgC: constant rows 8..15 of Eg (selector for the k-beta stream)
    EgC = np.zeros((8, 128), np.float32)
    for h in range(HEADS):
        EgC[h, h*AGENT:(h+1)*AGENT] = SCALE
    HB8 = EgC.copy()   # same pattern masks the gbar broadcast into Eg rows 0..7

    # DIAG: (9, 2, 128, 128) diagonal tap matrices (lhsT layout [K, M])
    dwc9 = dwc_w.reshape(c, 9)
    DIAG = np.zeros((9, 2, 128, 128), np.float32)
    for t in range(9):
        for pt in range(2):
            np.fill_diagonal(DIAG[t, pt], dwc9[pt*128:(pt+1)*128, t])

    BLK = np.zeros((16, 128), np.float32)                            # gbar -> (h,a) expand
    for a in range(16):
        BLK[a, a::16] = 1.0

    NEG9 = np.zeros((128, 18), np.float32)
    for pt in range(2):
        NEG9[:, pt*9:(pt+1)*9] = -dwc9[pt*128:(pt+1)*128, :]

    projb = np.stack([proj_b[:128], proj_b[128:]], axis=1).astype(np.float32)  # (128, 2)
    bvcol = np.stack([bv[:128], bv[128:]], axis=1).astype(np.float32)          # (128, 2)
    dwbcol = np.stack([dwc_b[:128], dwc_b[128:]], axis=1).astype(np.float32)   # (128, 2)

    flags = dict(
        has_qb=bool(np.any(q_b != 0)),
        has_kvb_v=bool(np.any(bv != 0)),
        has_dwcb=bool(np.any(dwc_b != 0)),
        has_projb=True,  # cheap, always fused in the final STT
    )

    LOGC_A = np.zeros((128, 128), np.float32)
    LOGC_A[8:16] = EgC
    LOGC_A[16:32] = an_tbl
    LOGC_A[32:96] = ahT
    LOGC_A[96:128] = awT[0:32]
    LOGC_B = awT[32:64].copy()                       # (32, 128)
    RHSC_A = np.concatenate([UU, Yind, Xind[0:32]], axis=0)   # (112, 4096)
    RHSC_B = Xind[32:64].copy()                               # (32, 4096)

    # SMALL_BF (128, 784): [ident 0:128 | blk 128:256 | hb8 256:384 | qrows 384:768 | i16 768:784]
    SMALL_BF = np.zeros((128, 784), np.float32)
    SMALL_BF[:, 0:128] = np.eye(128, dtype=np.float32)
    SMALL_BF[0:16, 128:256] = BLK
    SMALL_BF[0:8, 256:384] = HB8
    SMALL_BF[0:1, 384:768] = qrows
    SMALL_BF[0:16, 768:784] = np.eye(16, dtype=np.float32)
    # SMALL_F32 (128, 28): [neg9 0:18 | projb 18:20 | bvcol 20:22 | dwb 22:24 | posk1/k7 24:28]
    POS4 = np.stack([dwc9[0:128, 1], dwc9[128:256, 1],
                     dwc9[0:128, 7], dwc9[128:256, 7]], axis=1).astype(np.float32)
    SMALL_F32 = np.concatenate([NEG9, projb, bvcol, dwbcol, POS4], axis=1)

    params = dict(
        LOGC_A=LOGC_A.astype(BF), LOGC_B=LOGC_B.astype(BF),
        RHSC_A=RHSC_A.astype(BF), RHSC_B=RHSC_B.astype(BF),
        ABt=ABt.astype(BF), MM=MM, Wv=Wv,
        PW=proj_w.astype(np.float32).astype(BF),
        DIAG=DIAG.astype(BF), HM=headmask.astype(BF),
        SMALL_BF=SMALL_BF.astype(BF), SMALL_F32=SMALL_F32.astype(np.float32),
    )
    return params, flags


# ----------------------------------------------------------------------------
# device kernel builder
# ----------------------------------------------------------------------------

def _build(flags):
    nc = bacc.Bacc(None, target_bir_lowering=False, debug=False)

    # ---- DRAM I/O ----
    x_in = [nc.dram_tensor(f"x{m+1}", [DIM, N], F32, kind="ExternalInput") for m in range(2)]
    gblk = nc.dram_tensor("gblk", [16, 256], F32, kind="ExternalInput")
    gcols = nc.dram_tensor("gcols", [128, NCH], F32, kind="ExternalInput")
    dLOGA = nc.dram_tensor("LOGC_A", [128, 128], BF16, kind="ExternalInput")
    dLOGB = nc.dram_tensor("LOGC_B", [32, 128], BF16, kind="ExternalInput")
    dRHSA = nc.dram_tensor("RHSC_A", [112, N], BF16, kind="ExternalInput")
    dRHSB = nc.dram_tensor("RHSC_B", [32, N], BF16, kind="ExternalInput")
    dABt = nc.dram_tensor("ABt", [N, 128], BF16, kind="ExternalInput")
    dMM = nc.dram_tensor("MM", [DIM, 16], F32, kind="ExternalInput")
    dWv = nc.dram_tensor("Wv", [DIM, DIM], F32, kind="ExternalInput")
    dPW = nc.dram_tensor("PW", [DIM, DIM], BF16, kind="ExternalInput")
    dDIAG = nc.dram_tensor("DIAG", [9, 2, 128, 128], BF16, kind="ExternalInput")
    dHM = nc.dram_tensor("HM", [128, DIM], BF16, kind="ExternalInput")
    dSBF = nc.dram_tensor("SMALL_BF", [128, 784], BF16, kind="ExternalInput")
    dSF32 = nc.dram_tensor("SMALL_F32", [128, 28], F32, kind="ExternalInput")
    o_out = [nc.dram_tensor(f"o{m+1}", [DIM, N], F32, kind="ExternalOutput") for m in range(2)]

    with TileContext(nc) as tc:
        with (
            tc.tile_pool(name="wpool", bufs=1) as wp,          # weights/consts
            tc.tile_pool(name="big", bufs=1) as bigp,          # big per-branch tensors
            tc.tile_pool(name="xpool", bufs=2) as xp,          # input prefetch
            tc.tile_pool(name="small", bufs=3) as sp,          # rotating small tiles
            tc.tile_pool(name="ps_big", bufs=3, space="PSUM") as psb,    # (128,512)
            tc.tile_pool(name="ps_half", bufs=2, space="PSUM") as psh,   # (128,256)
            tc.tile_pool(name="ps_sm", bufs=2, space="PSUM") as pssm,    # (128,128)
            tc.tile_pool(name="ps_av", bufs=1, space="PSUM") as psav,
        ):
            # ---------------- critical-path DMAs first ----------------

            wv_f = sp.tile([128, 2 * DIM], F32, tag="ot", bufs=2)
            nc.sync.dma_start(wv_f[:, 0:DIM], dWv[0:128, :])
            nc.sync.dma_start(wv_f[:, DIM:2*DIM], dWv[128:256, :])
            wv = wp.tile([128, 2 * DIM], F32R)
            nc.vector.tensor_copy(wv[:], wv_f[:])

            mm_f = sp.tile([128, 2 * 16], F32, tag="mscratch", bufs=1)
            nc.sync.dma_start(mm_f[:, 0:16], dMM[0:128, :])
            nc.sync.dma_start(mm_f[:, 16:32], dMM[128:256, :])
            mmw = wp.tile([128, 2 * 16], F32R)
            nc.vector.tensor_copy(mmw[:], mm_f[:])

            xts = []
            rhsas = []
            for m in range(2):
                xtm = [xp.tile([128, N], F32, tag=f"x{m}{pt}", name=f"xt{m}{pt}", bufs=1) for pt in range(2)]
                if m == 0:
                    nc.sync.dma_start(xtm[0][:, 0:N//2], x_in[0][0:128, 0:N//2])
                    nc.sync.dma_start(xtm[0][:, N//2:N], x_in[0][0:128, N//2:N])
                    nc.sync.dma_start(xtm[1][:, 0:N//2], x_in[0][128:256, 0:N//2])
                    nc.sync.dma_start(xtm[1][:, N//2:N], x_in[0][128:256, N//2:N])
                else:
                    for pt in range(2):
                        nc.sync.dma_start(xtm[pt][:], x_in[m][pt*128:(pt+1)*128, :])
                xts.append(xtm)
                rhsa_m = bigp.tile([128, N], BF16, tag="rhsa", name=f"rhsa{m}", bufs=2)
                rhsas.append(rhsa_m)


            # ---------------- packed small consts ----------------
            smallbf = wp.tile([128, 784], BF16)
            nc.sync.dma_start(smallbf[:], dSBF[:])
            smallf = wp.tile([128, 28], F32)
            nc.sync.dma_start(smallf[:], dSF32[:])
            ident = smallbf[:, 0:128]
            blk = smallbf[0:16, 128:256]
            hb8 = smallbf[0:8, 256:384]
            qrows = smallbf[0:1, 384:768]
            i16 = smallbf[0:16, 768:784]
            neg9 = smallf[:, 0:18]
            projb = smallf[:, 18:20]
            bvcol = smallf[:, 20:22]
            dwbcol = smallf[:, 22:24]
            posk1 = smallf[:, 24:26]
            posk7 = smallf[:, 26:28]

            loga = wp.tile([128, 128], BF16)
            nc.sync.dma_start(loga[:], dLOGA[:])
            logb = wp.tile([32, 128], BF16)
            nc.sync.dma_start(logb[:], dLOGB[:])
            rhsb = wp.tile([32, N], BF16)
            nc.sync.dma_start(rhsb[:], dRHSB[:])
            abt = bigp.tile([128, NCH * 128], BF16, tag="attn", bufs=2)
            nc.sync.dma_start(
                abt[:].rearrange("p (j f) -> p j f", j=NCH),
                dABt[:].rearrange("(j p) f -> p j f", j=NCH))

            # ---------------- gbar & Eg ----------------
            gblk_t = wp.tile([16, 256], F32)
            nc.sync.dma_start(gblk_t[:], gblk[:])
            gsum = wp.tile([16, 1], F32)
            nc.vector.tensor_reduce(gsum[:], gblk_t[:], mybir.AxisListType.X, AL.add)
            gbar_col = wp.tile([16, 1], BF16)
            nc.vector.tensor_scalar(gbar_col[:], gsum[:], 1.0 / 256.0, None, AL.mult)

            ps_g2 = pssm.tile([1, 128], F32, tag="sm")
            nc.tensor.matmul(ps_g2[:], gbar_col[:], blk, start=True, stop=True)
            gbar128 = wp.tile([1, 128], BF16)
            nc.scalar.copy(gbar128[:], ps_g2[:])

            ones8 = wp.tile([1, 8], BF16)
            nc.vector.memset(ones8[:], 1.0)
            ps_e = pssm.tile([8, 128], F32, tag="sm")
            nc.tensor.matmul(ps_e[:], ones8[:], gbar128[:], start=True, stop=True)
            nc.vector.tensor_tensor(loga[0:8, :], ps_e[:], hb8, AL.mult)

            # ---------------- q-path rows ----------------
            u_row = wp.tile([1, 128], BF16)
            nc.vector.tensor_tensor(u_row[:], gbar128[:], qrows[:, 0:128], AL.mult)
            if flags["has_qb"]:
                nc.vector.tensor_tensor(u_row[:], u_row[:], qrows[:, 128:256], AL.add)
                r_row = wp.tile([1, 128], BF16)
                nc.vector.tensor_tensor(r_row[:], gbar128[:], qrows[:, 128:256], AL.mult)
                nc.vector.tensor_tensor(r_row[:], r_row[:], qrows[:, 256:384], AL.add)

            ones1 = wp.tile([1, 128], BF16)
            nc.vector.memset(ones1[:], 1.0)
            ps_u = pssm.tile([128, 128], F32, tag="sm")
            nc.tensor.matmul(ps_u[:], ones1[:], u_row[:], start=True, stop=True)
            u128 = wp.tile([128, 128], BF16)
            nc.scalar.copy(u128[:], ps_u[:])
            if flags["has_qb"]:
                ps_r = pssm.tile([128, 128], F32, tag="sm")
                nc.tensor.matmul(ps_r[:], ones1[:], r_row[:], start=True, stop=True)
                r128 = wp.tile([128, 128], BF16)
                nc.scalar.copy(r128[:], ps_r[:])

            gcols_t = wp.tile([128, NCH], F32)
            nc.sync.dma_start(gcols_t[:], gcols[:])

            hm = wp.tile([128, DIM], BF16)
            nc.sync.dma_start(hm[:], dHM[:])
            pw = wp.tile([128, 2 * DIM], BF16)   # PW as 2 K-half tiles side by side
            nc.sync.dma_start(pw[:, 0:DIM], dPW[0:128, :])
            nc.sync.dma_start(pw[:, DIM:2*DIM], dPW[128:256, :])
            diag = wp.tile([128, 18 * 128], BF16)
            nc.sync.dma_start(
                diag[:].rearrange("p (t m) -> p t m", t=18),
                dDIAG[:].rearrange("t q k m -> k (t q) m"))
            # ---------------- per-branch pipeline, phase-interleaved ----------------
            st = [dict(), dict()]
            diag3 = diag[:].rearrange("p (t m) -> p t m", t=18)
            qh = {}
            ps_av2 = psav.tile([128, 512], F32, tag="av")

            qnt_t = wp.tile([128, N], BF16, name="qnt")   # (h,a) x n, normalized q-attn

            def ph_qpath(g0, g1):
                qnt = qnt_t
                for grp in range(g0, g1):
                    lqg = sp.tile([128, 512], BF16, tag="lq", bufs=2, name="lqg")
                    for jj in range(4):
                        j = grp * 4 + jj
                        nc.vector.scalar_tensor_tensor(
                            lqg[:, jj*128:(jj+1)*128], u128[:], gcols_t[:, j:j+1],
                            abt[:, j*128:(j+1)*128], AL.mult, AL.add)
                        if flags["has_qb"]:
                            nc.vector.tensor_tensor(lqg[:, jj*128:(jj+1)*128],
                                                    lqg[:, jj*128:(jj+1)*128], r128[:], AL.add)
                    nc.scalar.activation(lqg[:], lqg[:], AF.Exp)
                    sqg = sp.tile([128, 32], F32, tag="sq", bufs=2, name="sqg")
                    nc.vector.tensor_reduce(sqg[:], lqg[:].rearrange("p (g b) -> p g b", b=16),
                                            mybir.AxisListType.X, AL.add)
                    rqg = sp.tile([128, 32], F32, tag="rq", bufs=2, name="rqg")
                    nc.vector.reciprocal(rqg[:], sqg[:])
                    qng = lqg
                    nc.vector.tensor_tensor(
                        qng[:].rearrange("p (g b) -> p g b", b=16),
                        qng[:].rearrange("p (g b) -> p g b", b=16),
                        rqg[:].unsqueeze(2).broadcast_to([128, 32, 16]), AL.mult)
                    for half in range(2):
                        ps_q = pssm.tile([128, 256], BF16, tag="sm", name="ps_q")
                        for q2 in range(2):
                            jj = half * 2 + q2
                            nc.tensor.transpose(ps_q[:, q2*128:(q2+1)*128],
                                                qng[:, jj*128:(jj+1)*128], ident)
                        base = (grp * 4 + half * 2) * 128
                        if (grp + half) % 2 == 0:
                            nc.scalar.copy(qnt[:, base:base+256], ps_q[:])
                        else:
                            nc.vector.tensor_copy(qnt[:, base:base+256], ps_q[:])
                qh["qnt"] = qnt

            for m in range(2):
                nc.sync.dma_start(rhsas[m][16:128, :], dRHSA[:])

            def ph_vt_kwf(m):
                xt = xts[m]
                vt = [bigp.tile([128, N], BF16, tag=f"vt{pt}", name=f"vt{m}{pt}", bufs=2) for pt in range(2)]
                rhsa = rhsas[m]
                for t in range(NT):
                    xrc = [sp.tile([128, NTW], F32R, tag=f"xrc{kh}", name=f"xrc{m}{kh}", bufs=2) for kh in range(2)]
                    (nc.gpsimd if (m + t) % 2 == 0 else nc.vector).tensor_copy(xrc[0][:], xt[0][:, t*NTW:(t+1)*NTW])
                    nc.gpsimd.tensor_copy(xrc[1][:], xt[1][:, t*NTW:(t+1)*NTW])
                    for pt in range(2):
                        ps_v = psb.tile([128, NTW], F32, tag="big")
                        for kh in range(2):
                            nc.tensor.matmul(
                                ps_v[:], wv[:, kh*DIM + pt*128: kh*DIM + (pt+1)*128],
                                xrc[kh][:],
                                start=(kh == 0), stop=(kh == 1))
                        sl = vt[pt][:, t*NTW:(t+1)*NTW]
                        if flags["has_kvb_v"]:
                            nc.vector.tensor_scalar(sl, ps_v[:], bvcol[:, pt:pt+1], None, AL.add)
                        else:
                            nc.scalar.copy(sl, ps_v[:])
                    ps_k = psh.tile([16, NTW], F32, tag="half")
                    for kh in range(2):
                        nc.tensor.matmul(
                            ps_k[:], mmw[:, kh*16:(kh+1)*16],
                            xrc[kh][:],
                            start=(kh == 0), stop=(kh == 1))
                    if m == 0 or t % 2 == 0:
                        nc.vector.tensor_copy(rhsa[0:16, t*NTW:(t+1)*NTW], ps_k[:])
                    else:
                        nc.scalar.copy(rhsa[0:16, t*NTW:(t+1)*NTW], ps_k[:])
                st[m]["vt"] = vt

            def ph_logits(m):
                rhsa = rhsas[m]
                attn = bigp.tile([128, N], BF16, tag="attn", name=f"attn{m}", bufs=2)
                s1p = bigp.tile([128, NT], F32, tag="s1p", name=f"s1p{m}", bufs=2)
                for t in range(NT):
                    ps_l = psb.tile([128, NTW], F32, tag="big")
                    nc.tensor.matmul(ps_l[:], loga[:], rhsa[:, t*NTW:(t+1)*NTW],
                                     start=True, stop=False)
                    nc.tensor.matmul(ps_l[:], logb[:], rhsb[:, t*NTW:(t+1)*NTW],
                                     start=False, stop=True)
                    nc.scalar.activation(attn[:, t*NTW:(t+1)*NTW], ps_l[:], AF.Exp,
                                         accum_out=s1p[:, t:t+1])
                s1 = bigp.tile([128, 1], F32, tag="s1", name=f"s1{m}", bufs=2)
                nc.vector.tensor_reduce(s1[:], s1p[:], mybir.AxisListType.X, AL.add)
                rs1 = bigp.tile([128, 1], F32, tag="rs1", name=f"rs1{m}", bufs=2)
                nc.vector.reciprocal(rs1[:], s1[:])
                st[m]["attn"] = attn
                st[m]["rs1"] = rs1

            def ph_transp_av(m):
                attn, vt, rs1 = st[m]["attn"], st[m]["vt"], st[m]["rs1"]
                ps_av = ps_av2[:, m*DIM:(m+1)*DIM]
                for kp in range(NCH // 2):
                    ps_t = psh.tile([128, 768], BF16, tag="half")
                    for q in range(2):
                        k = kp * 2 + q
                        nc.tensor.transpose(ps_t[:, q*384:q*384 + 128],
                                            attn[:, k*128:(k+1)*128], ident)
                        for pt in range(2):
                            nc.tensor.transpose(
                                ps_t[:, q*384 + 128 + pt*128:q*384 + 128 + (pt+1)*128],
                                vt[pt][:, k*128:(k+1)*128], ident)
                    tv = sp.tile([128, 768], BF16, tag="tv", name=f"tv{m}", bufs=2)
                    if m == 0:
                        nc.vector.tensor_copy(tv[:], ps_t[:])
                    else:
                        nc.scalar.copy(tv[:], ps_t[:])
                    for q in range(2):
                        k = kp * 2 + q
                        nc.tensor.matmul(ps_av[:], tv[:, q*384:q*384 + 128],
                                         tv[:, q*384 + 128:q*384 + 384],
                                         start=(k == 0), stop=(k == NCH - 1))
                avsel = bigp.tile([128, DIM], BF16, tag="avsel", name=f"avsel{m}", bufs=2)
                nc.vector.scalar_tensor_tensor(avsel[:], ps_av[:], rs1[:], hm[:],
                                               AL.mult, AL.mult)
                st[m]["avsel"] = avsel

            def ph_tail(m):
                vt, avsel, xt = st[m]["vt"], st[m]["avsel"], xts[m]
                qnt = qh["qnt"]
                pre = [bigp.tile([128, N], BF16, tag=f"pre{pt}", name=f"pre{m}{pt}", bufs=2) for pt in range(2)]
                pre3 = [pre[pt][:].rearrange("p (y x) -> p y x", y=H) for pt in range(2)]
                vt3 = [vt[pt][:].rearrange("p (y x) -> p y x", y=H) for pt in range(2)]
                CORR_HI = ((-1, 2, 0), (0, 5, 1), (1, 8, 2))   # (dy, k, ysrc_off): v[y+off,0] -> pre[y,63]
                CORR_LO = ((-1, 0, -2), (0, 3, -1), (1, 6, 0))  # v[y+off,63] -> pre[y,0]
                for t in range(NT):
                    r0 = t * ROWS_PER_NT
                    for pt in range(2):
                        ps_n = psb.tile([128, NTW], F32, tag="big")
                        taps = []
                        tap = 0
                        for dy in (-1, 0, 1):
                            a0, b0 = max(0, -dy), H - max(0, dy)
                            for dx in (-1, 0, 1):
                                s = dy * W + dx
                                lo = max(t * NTW, a0 * W, -s)
                                hi = min((t + 1) * NTW, b0 * W, N - s)
                                taps.append((tap, lo, hi, s))
                                tap += 1
                        taps.sort(key=lambda q: q[0] != 4)   # center tap first
                        taps = [q for q in taps if q[0] not in (1, 7)]
                        for tap, lo, hi, s in taps:
                            nc.tensor.matmul(
                                ps_n[:, lo - t*NTW:hi - t*NTW],
                                diag3[:, tap*2 + pt, :],
                                vt[pt][:, lo + s:hi + s],
                                start=(tap == 4), stop=False,
                                skip_group_check=True)
                        nc.tensor.matmul(ps_n[:],
                                         avsel[:, pt*128:(pt+1)*128],
                                         qnt[:, t*NTW:(t+1)*NTW],
                                         start=False, stop=True,
                                         skip_group_check=True)
                        sl = pre[pt][:, t*NTW:(t+1)*NTW]
                        if flags["has_dwcb"]:
                            nc.vector.tensor_scalar(sl, ps_n[:],
                                                    dwbcol[:, pt:pt+1], None, AL.add)
                        else:
                            nc.scalar.copy(sl, ps_n[:])
                        # dx=0 taps as SBUF-only in-place DVE ops (2x mode)
                        lo7, hi7 = t*NTW, min((t+1)*NTW, N - W)
                        nc.vector.scalar_tensor_tensor(
                            pre[pt][:, lo7:hi7], vt[pt][:, lo7+W:hi7+W],
                            posk7[:, pt:pt+1], pre[pt][:, lo7:hi7], AL.mult, AL.add)
                        lo1, hi1 = max(t*NTW, W), (t+1)*NTW
                        nc.vector.scalar_tensor_tensor(
                            pre[pt][:, lo1:hi1], vt[pt][:, lo1-W:hi1-W],
                            posk1[:, pt:pt+1], pre[pt][:, lo1:hi1], AL.mult, AL.add)
                        # x-wraparound border corrections, rows of this n-tile only
                        for dy, k, off in CORR_HI:
                            ya = max(r0, max(0, -dy), -off)
                            yb = min(r0 + ROWS_PER_NT, H - max(0, dy), H - off)
                            if yb > ya:
                                nc.vector.scalar_tensor_tensor(
                                    pre3[pt][:, ya:yb, 63:64],
                                    vt3[pt][:, ya + off:yb + off, 0:1],
                                    neg9[:, pt*9 + k:pt*9 + k + 1],
                                    pre3[pt][:, ya:yb, 63:64],
                                    AL.mult, AL.add)
                        for dy, k, off in CORR_LO:
                            ya = max(r0, max(0, -dy), -off, 1 - dy)
                            yb = min(r0 + ROWS_PER_NT, H - max(0, dy), H - off)
                            if yb > ya:
                                nc.vector.scalar_tensor_tensor(
                                    pre3[pt][:, ya:yb, 0:1],
                                    vt3[pt][:, ya + off:yb + off, 63:64],
                                    neg9[:, pt*9 + k:pt*9 + k + 1],
                                    pre3[pt][:, ya:yb, 0:1],
                                    AL.mult, AL.add)
                    for mt in range(2):
                        ps_o = psh.tile([128, NTW], F32, tag="half")
                        for kh in range(2):
                            nc.tensor.matmul(
                                ps_o[:], pw[:, kh*DIM + mt*128: kh*DIM + (mt+1)*128],
                                pre[kh][:, t*NTW:(t+1)*NTW],
                                start=(kh == 0), stop=(kh == 1))
                        ot = sp.tile([128, NTW], F32, tag="ot", name=f"ot{m}", bufs=2)
                        nc.vector.scalar_tensor_tensor(
                            ot[:], ps_o[:], projb[:, mt:mt+1],
                            xt[mt][:, t*NTW:(t+1)*NTW], AL.add, AL.add)
                        nc.sync.dma_start(o_out[m][mt*128:(mt+1)*128, t*NTW:(t+1)*NTW], ot[:])

            ph_qpath(0, 2)
            ph_vt_kwf(0)
            ph_vt_kwf(1)
            ph_qpath(2, 8)
            ph_logits(0)
            ph_transp_av(0)
            ph_logits(1)
            ph_transp_av(1)
            ph_tail(0)
            ph_tail(1)

    nc.compile()
    return nc


# ----------------------------------------------------------------------------
# public entry point
# ----------------------------------------------------------------------------

_CACHE = {}


def kernel(**inputs):
    inputs = {k: np.asarray(v) for k, v in inputs.items()}
    params, flags = _host_precompute(
        **{k: inputs[k] for k in
           ("kv_w", "kv_b", "q_w", "q_b", "proj_w", "proj_b", "dwc_w", "dwc_b",
            "an_bias", "na_bias", "ah_bias", "aw_bias", "ha_bias", "wa_bias")})

    key = tuple(sorted(flags.items()))
    if key not in _CACHE:
        _CACHE[key] = _build(flags)
    nc = _CACHE[key]

    in_maps = _make_in_maps(inputs, params)

    res = run_bass_kernel_spmd(nc, in_maps, core_ids=list(range(B)))
    o1 = np.stack([res.results[b]["o1"].reshape(DIM, H, W) for b in range(B)])
    o2 = np.stack([res.results[b]["o2"].reshape(DIM, H, W) for b in range(B)])
    return o1.astype(np.float32), o2.astype(np.float32)


def _make_in_maps(inputs, params):
    input1, input2, guidmap = inputs["input1"], inputs["input2"], inputs["guidmap"]
    shared = {
        "LOGC_A": params["LOGC_A"], "LOGC_B": params["LOGC_B"],
        "RHSC_A": params["RHSC_A"], "RHSC_B": params["RHSC_B"],
        "ABt": params["ABt"], "MM": params["MM"],
        "Wv": params["Wv"], "PW": params["PW"], "DIAG": params["DIAG"],
        "HM": params["HM"], "SMALL_BF": params["SMALL_BF"], "SMALL_F32": params["SMALL_F32"],
    }
    in_maps = []
    for b in range(B):
        g = guidmap[b].reshape(N).astype(np.float32)
        gimg = g.reshape(H, W)
        gblk = gimg.reshape(PS, AGENT, PS, AGENT).transpose(0, 2, 1, 3).reshape(AGENT, 256)
        gcols = g.reshape(NCH, 128).T.copy()
        in_maps.append({
            "x1": np.ascontiguousarray(input1[b].reshape(DIM, N)),
            "x2": np.ascontiguousarray(input2[b].reshape(DIM, N)),
            "gblk": np.ascontiguousarray(gblk.astype(np.float32)),
            "gcols": np.ascontiguousarray(gcols.astype(np.float32)),
            **shared,
        })
    return in_maps

